# revision 8
# baseline (speedup 1.0000x reference)
"""Distortion-loss (eff_distloss) Bass kernel for Trainium2, 8 NeuronCores.

Inputs (full): weights/distances/intervals, each [262144, 128] f32.
Output: scalar f32 loss.

Math: per ray (w, m, s in R^128):
  uni = sum_j s_j w_j^2
  bi  = sum_{j>k} w_j w_k (m_j - m_k) = wm^T (SL - SU) w,  wm = w*m,
        SL/SU strictly lower/upper triangular ones.
  loss = 0.01 * mean_rays(uni/3 + 2*bi)

Total bi over a batch of rays = <A^T, W^T WM>_F with A = SL - SU (constant)
and W^T WM a Gram matrix accumulated over rays; uni = sum diag(W^T SW),
sw = s*w. On the PE, each 128-ray block is ONE ldweights (stationary w) +
ONE 256-wide matmul streaming [wm | sw] into a single [128, 256] PSUM
accumulator holding both Gram matrices side by side. The 2.0 (bi) and 1/3
(uni) loss weights are folded into the constant [2A | I/3] matrix, so the
finale is one mul + row-sum producing a single partial per partition,
followed by a DVE stream-transpose that folds the 128 partials into 4
partitions x 32 lanes (a [P,2] out-DMA costs ~3us in 8-byte descriptor
processing; [4,32] is 4 packets).

Sharding: pure data-parallel over the ray axis, B=262144 -> 32768 rays on
each of the 8 cores; the host sums 128 partials per core and scales.

Raw-bass implementation (no Tile). DMA tiling is 32 rays/partition so
each descriptor is a 16 KiB contiguous run — the DMA descriptor-ring
processor (engine 79) saturates before HBM does with 8 KiB descriptors.
Compute runs at 8-ray chunk granularity within each DMA tile so the
ACT/DVE/PE pipeline keeps fine-grained overlap. Engine split: sync
issues all input DMAs; the scalar (ACT) engine does every f32->bf16
weight cast; DVE does the wm/sw products and the finale; gpsimd does the
last tile's wm quarters; PE does the Gram matmuls. Steady-state tiles use
one semaphore per ring slot at full-tile thresholds (a counting
semaphore shared by interleaved multi-engine DMAs is only sound at its
final total). The last four 8-ray tiles stream with one semaphore per
DMA in stream-major order, the final tile's m/s streams quartered, and
aimat last, so when the final s bytes land only one small product, two
matmuls and the finale remain.
"""

import numpy as np

import concourse.bass as bass
import concourse.mybir as mybir
from concourse.bass_utils import run_bass_kernel_spmd

B, N = 262144, 128
NCORES = 8
B_PER = B // NCORES  # 32768 rays per core
P = 128  # SBUF partitions = rays per matmul block
RMAX = 32  # rays per partition in a full DMA tile (16 KiB descriptors)
# 7 full DMA tiles + 4 half compute tiles = 7*32 + 4*8 = 256 ray-blocks
SCHED = [32] * 7 + [8, 8, 8, 8]
assert sum(SCHED) * P == B_PER
T = len(SCHED)
NTAIL = 4  # tiles T-4..T-1: stream-major with per-DMA semaphores
CR = 8  # compute chunk = 8 rays/partition (1024 f32 els)
FREE = RMAX * N  # ring slot size (f32 elements per partition)
NB = 2  # ring depth
NQ = 4  # last-tile m/s stream + compute split

F32 = mybir.dt.float32
BF16 = mybir.dt.bfloat16

LOSS_WEIGHT = 0.01

_cached = {}


def _build_nc() -> bass.Bass:
    nc = bass.Bass(trn_type="TRN2", monotonic_sem_count=0)

    w_h = nc.declare_dram_parameter("weights", [B_PER, N], F32, isOutput=False)
    m_h = nc.declare_dram_parameter("distances", [B_PER, N], F32, isOutput=False)
    s_h = nc.declare_dram_parameter("intervals", [B_PER, N], F32, isOutput=False)
    ai_h = nc.declare_dram_parameter("aimat", [P, 2 * N], F32, isOutput=False)
    out_h = nc.declare_dram_parameter("partials", [4, 32], F32, isOutput=True)

    # per-tile DRAM views: tile i covers rays [off, off + P*R_i)
    offs = [0]
    for r in SCHED:
        offs.append(offs[-1] + P * r)

    def dram_view(h, i):
        r = SCHED[i]
        return h[offs[i] : offs[i + 1], :].rearrange("(p r) n -> p (r n)", p=P, r=r)

    TM = T - NTAIL  # DMA tiles 0..TM-1 in the steady-state loop (slot-sem)
    # compute chunks: (tile, chunk-within-tile); main tiles have
    # SCHED[i]//CR chunks, tail tiles 0..2 have one, tile T-1 is quartered
    CPT = [SCHED[i] // CR for i in range(TM)]  # 4 chunks per main tile
    NCH = sum(CPT)  # 28 main chunks
    # ACT increments: 1 cast per main chunk + 1 per tail tile
    ACT_T = NCH + NTAIL  # 32
    # DVE increments: 2 per main chunk (wm, sw); tail: wm+sw for tail
    # tiles 0..2, sw quarters for the last, reduce, transpose
    DVE_MAIN = 2 * NCH  # 56
    DVE_FINAL = DVE_MAIN + 2 * (NTAIL - 1) + NQ + 2  # 68
    # PE increments: one per main chunk, per tail tile 0..2, + final stop
    PE_DONE = NCH + NTAIL  # 32

    def chunks_done(i):
        # total main chunks in tiles 0..i-1 (i <= TM)
        return sum(CPT[:i])

    def pe_tile_done(i):
        # pe_sem value once tile i is fully consumed by the PE
        if i < TM:
            return chunks_done(i + 1)
        return NCH + (i - TM + 1)

    def act_tile_done(i):
        if i < TM:
            return chunks_done(i + 1)
        return NCH + (i - TM + 1)

    def dve_tile_done(i):
        if i < TM:
            return 2 * chunks_done(i + 1)
        j = i - TM
        if j < NTAIL - 1:
            return DVE_MAIN + 2 * (j + 1)
        return DVE_MAIN + 2 * (NTAIL - 1) + NQ

    R_LAST = SCHED[-1]
    QF = R_LAST * N // NQ  # f32 elements per quarter of the last tile
    QR = R_LAST // NQ  # ray-blocks per quarter
    CF = CR * N  # f32 elements per compute chunk

    import contextlib

    with contextlib.ExitStack() as ctx:
        ec = ctx.enter_context
        w_sb = ec(nc.sbuf_tensor([P, NB * FREE], F32))
        m_sb = ec(nc.sbuf_tensor([P, NB * FREE], F32))
        s_sb = ec(nc.sbuf_tensor([P, NB * FREE], F32))
        # [wm | sw] interleaved per ray block: block r occupies columns
        # [r*2N, r*2N + 2N) of the slot, wm in the low half, sw in the high
        ws_sb = ec(nc.sbuf_tensor([P, NB * 2 * FREE], BF16))
        wb_sb = ec(nc.sbuf_tensor([P, NB * FREE], BF16))
        ai_sb = ec(nc.sbuf_tensor([P, 2 * N], F32))
        tr_sb = ec(nc.sbuf_tensor([P, 2 * N], F32))
        pad_sb = ec(nc.sbuf_tensor([P, 32], F32))
        tp_sb = ec(nc.sbuf_tensor([P, 32], F32))
        warm_sb = ec(nc.sbuf_tensor([P, 32], BF16))
        g12_ps = ec(nc.psum_tensor([P, 2 * N], F32))  # [W^T WM | W^T SW]
        slot_sem = [ec(nc.semaphore(f"dma_slot{i}")) for i in range(NB)]
        # tail: one semaphore per DMA so every wait is at a final total
        tw_sem = [ec(nc.semaphore(f"dma_w{j}")) for j in range(NTAIL)]
        tm_sem = [ec(nc.semaphore(f"dma_m{j}")) for j in range(NTAIL - 1)]
        ts_sem = [ec(nc.semaphore(f"dma_s{j}")) for j in range(NTAIL - 1)]
        tmq_sem = [ec(nc.semaphore(f"dma_mq{q}")) for q in range(NQ)]
        tsq_sem = [ec(nc.semaphore(f"dma_sq{q}")) for q in range(NQ)]
        ai_sem = ec(nc.semaphore("dma_ai"))
        act_sem = ec(nc.semaphore("act_sem"))
        gp_sem = ec(nc.semaphore("gp_sem"))
        dve_sem = ec(nc.semaphore("dve_sem"))
        pe_sem = ec(nc.semaphore("pe_sem"))
        block = ec(nc.Block(no_gpsimd_drain=True))

        def sl(i, n_el=None):
            base = (i % NB) * FREE
            return slice(base, base + (SCHED[i] * N if n_el is None else n_el))

        def csl(i, c):
            # compute-chunk c of tile i (CF elements)
            base = (i % NB) * FREE
            return slice(base + c * CF, base + (c + 1) * CF)

        def f32_3d(t_sb, i, c=None, q=None):
            # [P, R, N] view of an io slot / chunk / last-tile quarter
            if q is not None:
                base = (i % NB) * FREE
                v = t_sb[:, base + q * QF : base + (q + 1) * QF]
            elif c is not None:
                v = t_sb[:, csl(i, c)]
            else:
                v = t_sb[:, sl(i)]
            return v.rearrange("p (r n) -> p r n", n=N)

        def ws_3d(i, half, c=None, q=None):
            # [P, R, N] strided view into the [wm | sw] pair layout
            base2 = (i % NB) * 2 * FREE
            if q is not None:
                v = ws_sb[:, base2 + q * 2 * QF : base2 + (q + 1) * 2 * QF]
            elif c is not None:
                v = ws_sb[:, base2 + c * 2 * CF : base2 + (c + 1) * 2 * CF]
            else:
                r = SCHED[i]
                v = ws_sb[:, base2 : base2 + 2 * r * N]
            v = v.rearrange("p (r x) -> p r x", x=2 * N)
            return v[:, :, half * N : (half + 1) * N]

        @block.sync
        def _(sync: bass.BassEngine):
            for i in range(TM):
                k = i % NB
                if i >= NB:
                    # io ring slot (i-NB): w consumed by ACT cast + DVE
                    # wm/sw; m/s consumed by DVE
                    sync.wait_ge(act_sem, act_tile_done(i - NB))
                    sync.wait_ge(dve_sem, dve_tile_done(i - NB))
                sync.dma_start(out=w_sb[:, sl(i)], in_=dram_view(w_h, i)).then_inc(
                    slot_sem[k], 16
                )
                sync.dma_start(out=m_sb[:, sl(i)], in_=dram_view(m_h, i)).then_inc(
                    slot_sem[k], 16
                )
                sync.dma_start(out=s_sb[:, sl(i)], in_=dram_view(s_h, i)).then_inc(
                    slot_sem[k], 16
                )
            # stream-major tail
            for j in range(NTAIL):
                i = TM + j
                sync.wait_ge(act_sem, act_tile_done(i - NB))
                sync.wait_ge(dve_sem, dve_tile_done(i - NB))
                sync.dma_start(out=w_sb[:, sl(i)], in_=dram_view(w_h, i)).then_inc(
                    tw_sem[j], 16
                )
                if j < NTAIL - 1:
                    sync.dma_start(out=m_sb[:, sl(i)], in_=dram_view(m_h, i)).then_inc(
                        tm_sem[j], 16
                    )
                    sync.dma_start(out=s_sb[:, sl(i)], in_=dram_view(s_h, i)).then_inc(
                        ts_sem[j], 16
                    )
            # final tile: m and s quartered; aimat dead last (the finale
            # needs it only after the last matmul)
            i = T - 1
            base = (i % NB) * FREE
            m_last = dram_view(m_h, i)
            s_last = dram_view(s_h, i)
            for q in range(NQ):
                sync.dma_start(
                    out=m_sb[:, base + q * QF : base + (q + 1) * QF],
                    in_=m_last[:, q * QF : (q + 1) * QF],
                ).then_inc(tmq_sem[q], 16)
            for q in range(NQ):
                sync.dma_start(
                    out=s_sb[:, base + q * QF : base + (q + 1) * QF],
                    in_=s_last[:, q * QF : (q + 1) * QF],
                ).then_inc(tsq_sem[q], 16)
            sync.dma_start(out=ai_sb[:], in_=ai_h[:, :]).then_inc(ai_sem, 16)
            sync.wait_ge(dve_sem, DVE_FINAL)
            sync.dma_start(out=out_h[:, :], in_=tp_sb[0:P:32, 0:32]).then_inc(
                pe_sem, 16
            )
            # the out-DMA must fully land before the NEFF ends: an in-flight
            # DMA across the NEFF boundary corrupts runtime state.
            sync.wait_ge(pe_sem, PE_DONE + 16)

        @block.scalar
        def _(scalar: bass.BassEngine):
            # warmup: first ACTIVATE triggers a ~5us ACT table load; eat it
            # here instead of on tile 0's cast (value is irrelevant; wb_sb
            # is only written later by this same engine)
            scalar.activation(
                warm_sb[:, :], wb_sb[:, 0:32], mybir.ActivationFunctionType.Copy
            )
            for i in range(TM):
                k = i % NB
                scalar.wait_ge(slot_sem[k], 48 * (i // NB + 1))
                if i >= NB:
                    # bf16 ring slot (i-NB) fully consumed by PE
                    scalar.wait_ge(pe_sem, pe_tile_done(i - NB))
                for c in range(CPT[i]):
                    scalar.activation(
                        wb_sb[:, csl(i, c)],
                        w_sb[:, csl(i, c)],
                        mybir.ActivationFunctionType.Copy,
                    ).then_inc(act_sem, 1)
            for j in range(NTAIL):
                i = TM + j
                scalar.wait_ge(tw_sem[j], 16)
                scalar.wait_ge(pe_sem, pe_tile_done(i - NB))
                scalar.activation(
                    wb_sb[:, sl(i)],
                    w_sb[:, sl(i)],
                    mybir.ActivationFunctionType.Copy,
                ).then_inc(act_sem, 1)

        @block.gpsimd
        def _(gpsimd: bass.BassEngine):
            # last tile's wm products, quarter by quarter (m stream chase)
            i = T - 1
            gpsimd.wait_ge(tw_sem[NTAIL - 1], 16)
            gpsimd.wait_ge(pe_sem, pe_tile_done(i - NB))
            for q in range(NQ):
                gpsimd.wait_ge(tmq_sem[q], 16)
                gpsimd.tensor_mul(
                    ws_3d(i, 0, q=q), f32_3d(w_sb, i, q=q), f32_3d(m_sb, i, q=q)
                ).then_inc(gp_sem, 1)

        @block.vector
        def _(vector: bass.BassEngine):
            # zero the transpose staging pad once (cols 1..31 stay zero)
            vector.memset(pad_sb[:, :], 0.0)
            for i in range(TM):
                k = i % NB
                vector.wait_ge(slot_sem[k], 48 * (i // NB + 1))
                if i >= NB:
                    vector.wait_ge(pe_sem, pe_tile_done(i - NB))
                for c in range(CPT[i]):
                    vector.tensor_mul(
                        ws_3d(i, 0, c), f32_3d(w_sb, i, c), f32_3d(m_sb, i, c)
                    ).then_inc(dve_sem, 1)
                    vector.tensor_mul(
                        ws_3d(i, 1, c), f32_3d(s_sb, i, c), f32_3d(w_sb, i, c)
                    ).then_inc(dve_sem, 1)
            for j in range(NTAIL - 1):
                i = TM + j
                vector.wait_ge(pe_sem, pe_tile_done(i - NB))
                vector.wait_ge(tw_sem[j], 16)
                vector.wait_ge(tm_sem[j], 16)
                vector.tensor_mul(
                    ws_3d(i, 0), f32_3d(w_sb, i), f32_3d(m_sb, i)
                ).then_inc(dve_sem, 1)
                vector.wait_ge(ts_sem[j], 16)
                vector.tensor_mul(
                    ws_3d(i, 1), f32_3d(s_sb, i), f32_3d(w_sb, i)
                ).then_inc(dve_sem, 1)
            # last tile: chase the s quarters (wm quarters run on gpsimd)
            i = T - 1
            vector.wait_ge(pe_sem, pe_tile_done(i - NB))
            vector.wait_ge(tw_sem[NTAIL - 1], 16)
            for q in range(NQ):
                vector.wait_ge(tsq_sem[q], 16)
                vector.tensor_mul(
                    ws_3d(i, 1, q=q), f32_3d(s_sb, i, q=q), f32_3d(w_sb, i, q=q)
                ).then_inc(dve_sem, 1)
            # finale: (G12 * [2A | I/3]) row-sum -> 4x32 fold.
            # drain: the stream-shuffle transpose does not interlock with
            # the preceding ALU op's writeback.
            vector.wait_ge(pe_sem, PE_DONE)
            vector.wait_ge(ai_sem, 16)
            vector.tensor_mul(tr_sb[:], g12_ps[:], ai_sb[:])
            vector.tensor_reduce(
                pad_sb[:, 0:1],
                tr_sb[:],
                axis=mybir.AxisListType.X,
                op=mybir.AluOpType.add,
            ).then_inc(dve_sem, 1)
            vector.drain()
            vector.transpose(out=tp_sb[:, :], in_=pad_sb[:, :]).then_inc(dve_sem, 1)

        @block.tensor
        def _(tensor: bass.BassEngine):
            def blk_mms(i, blocks, start=False, stop=False):
                base = (i % NB) * FREE
                base2 = (i % NB) * 2 * FREE
                last_mm = None
                for idx, r in enumerate(blocks):
                    wblk = slice(base + r * N, base + (r + 1) * N)
                    pblk = slice(base2 + r * 2 * N, base2 + (r + 1) * 2 * N)
                    last_mm = nc.tensor.matmul(
                        out=g12_ps[:],
                        lhsT=wb_sb[:, wblk],
                        rhs=ws_sb[:, pblk],
                        start=(start and idx == 0),
                        stop=(stop and idx == len(blocks) - 1),
                    )
                return last_mm

            ch = 0
            for i in range(TM):
                for c in range(CPT[i]):
                    ch += 1
                    tensor.wait_ge(act_sem, ch)
                    tensor.wait_ge(dve_sem, 2 * ch)
                    blk_mms(
                        i,
                        range(c * CR, (c + 1) * CR),
                        start=(ch == 1),
                    ).then_inc(pe_sem, 1)
            for j in range(NTAIL - 1):
                i = TM + j
                tensor.wait_ge(act_sem, NCH + j + 1)
                tensor.wait_ge(dve_sem, DVE_MAIN + 2 * (j + 1))
                blk_mms(i, range(SCHED[i])).then_inc(pe_sem, 1)
            i = T - 1
            tensor.wait_ge(act_sem, ACT_T)
            last_mm = None
            for q in range(NQ):
                tensor.wait_ge(gp_sem, q + 1)
                tensor.wait_ge(dve_sem, DVE_MAIN + 2 * (NTAIL - 1) + q + 1)
                last_mm = blk_mms(
                    i, range(q * QR, (q + 1) * QR), stop=(q == NQ - 1)
                )
            last_mm.then_inc(pe_sem, 1)

    return nc


def _a2mat() -> np.ndarray:
    # transpose of (SL - SU): the kernel accumulates W^T WM = G1^T, and
    # <A, G1> = <A^T, G1^T>
    a = np.triu(np.ones((N, N), np.float32), 1) - np.tril(
        np.ones((N, N), np.float32), -1
    )
    return np.ascontiguousarray(a, dtype=np.float32)


def _aimat() -> np.ndarray:
    # loss weights folded in: [2A | I/3]
    return np.ascontiguousarray(
        np.concatenate(
            [2.0 * _a2mat(), np.eye(N, dtype=np.float32) / 3.0], axis=1
        )
    )


def kernel(weights: np.ndarray, distances: np.ndarray, intervals: np.ndarray):
    if "nc" not in _cached:
        _cached["nc"] = _build_nc()
    nc = _cached["nc"]

    w8 = np.ascontiguousarray(weights, np.float32).reshape(NCORES, B_PER, N)
    m8 = np.ascontiguousarray(distances, np.float32).reshape(NCORES, B_PER, N)
    s8 = np.ascontiguousarray(intervals, np.float32).reshape(NCORES, B_PER, N)
    ai = _aimat()

    in_maps = [
        {
            "weights": w8[i],
            "distances": m8[i],
            "intervals": s8[i],
            "aimat": ai,
        }
        for i in range(NCORES)
    ]
    res = run_bass_kernel_spmd(nc, in_maps, list(range(NCORES))).results

    total = 0.0
    for i in range(NCORES):
        total += res[i]["partials"].astype(np.float64).sum()

    loss = LOSS_WEIGHT * total / B
    return np.asarray(loss, dtype=np.float32)


# revision 9
# speedup vs baseline: 1.0862x; 1.0862x over previous
"""Distortion-loss (eff_distloss) Bass kernel for Trainium2, 8 NeuronCores.

Inputs (full): weights/distances/intervals, each [262144, 128] f32.
Output: scalar f32 loss.

Math: per ray (w, m, s in R^128):
  uni = sum_j s_j w_j^2
  bi  = sum_{j>k} w_j w_k (m_j - m_k) = wm^T (SL - SU) w,  wm = w*m,
        SL/SU strictly lower/upper triangular ones.
  loss = 0.01 * mean_rays(uni/3 + 2*bi)

Total bi over a batch of rays = <A^T, W^T WM>_F with A = SL - SU (constant)
and W^T WM a Gram matrix accumulated over rays; uni = sum diag(W^T SW),
sw = s*w. On the PE, each 128-ray block is ONE ldweights (stationary w) +
ONE 256-wide matmul streaming [wm | sw] into a single [128, 256] PSUM
accumulator holding both Gram matrices side by side. The 2.0 (bi) and 1/3
(uni) loss weights are folded into the constant [2A | I/3] matrix, so the
finale is one mul + row-sum producing a single partial per partition,
followed by a DVE stream-transpose that folds the 128 partials into 4
partitions x 32 lanes (a [P,2] out-DMA costs ~3us in 8-byte descriptor
processing; [4,32] is 4 packets).

Sharding: pure data-parallel over the ray axis, B=262144 -> 32768 rays on
each of the 8 cores; the host sums 128 partials per core and scales.

Raw-bass implementation (no Tile). DMA tiles are 16 rays/partition (8 KiB
descriptors — the DMA engines' sweet spot; 16 KiB descriptors measurably
drop per-engine throughput) with an NB=4 ring so the DMA stream runs
3 tiles ahead of compute. Engine split: sync issues all input DMAs; the
scalar (ACT) engine does every f32->bf16 weight cast; DVE does the wm/sw
products and the finale; gpsimd does the last tile's wm quarters; PE does
the Gram matmuls. Steady-state tiles 0..13 use one semaphore per ring
slot at full-tile thresholds (a counting semaphore shared by interleaved
multi-engine DMAs is only sound at its final total). The last four 8-ray
tiles stream with one semaphore per DMA in stream-major order, the final
tile's m/s streams quartered, and aimat last, so when the final s bytes
land only one small product, two matmuls and the finale remain.
"""

import numpy as np

import concourse.bass as bass
import concourse.mybir as mybir
from concourse.bass_utils import run_bass_kernel_spmd

B, N = 262144, 128
NCORES = 8
B_PER = B // NCORES  # 32768 rays per core
P = 128  # SBUF partitions = rays per matmul block
RMAX = 16  # rays per partition in a full tile
# 14 full tiles + 4 half tiles = 14*16 + 4*8 = 256 ray-blocks per core
SCHED = [16] * 14 + [8, 8, 8, 8]
assert sum(SCHED) * P == B_PER
T = len(SCHED)
NTAIL = 4  # tiles T-4..T-1: stream-major with per-DMA semaphores
FREE = RMAX * N  # ring slot size (f32 elements per partition)
NB = 4  # ring depth
NQ = 4  # last-tile m/s stream + compute split

F32 = mybir.dt.float32
BF16 = mybir.dt.bfloat16

LOSS_WEIGHT = 0.01

_cached = {}


def _build_nc() -> bass.Bass:
    nc = bass.Bass(trn_type="TRN2", monotonic_sem_count=0)

    w_h = nc.declare_dram_parameter("weights", [B_PER, N], F32, isOutput=False)
    m_h = nc.declare_dram_parameter("distances", [B_PER, N], F32, isOutput=False)
    s_h = nc.declare_dram_parameter("intervals", [B_PER, N], F32, isOutput=False)
    ai_h = nc.declare_dram_parameter("aimat", [P, 2 * N], F32, isOutput=False)
    out_h = nc.declare_dram_parameter("partials", [4, 32], F32, isOutput=True)

    # per-tile DRAM views: tile i covers rays [off, off + P*R_i)
    offs = [0]
    for r in SCHED:
        offs.append(offs[-1] + P * r)

    def dram_view(h, i):
        r = SCHED[i]
        return h[offs[i] : offs[i + 1], :].rearrange("(p r) n -> p (r n)", p=P, r=r)

    TM = T - NTAIL  # tiles 0..TM-1 in the steady-state loop (slot-sem)
    # DVE increments: 2 per steady tile (wm, sw); tail: wm+sw for tiles
    # 14..16, sw quarters for tile 17, reduce, transpose
    DVE_MAIN = 2 * TM  # 28
    DVE_FINAL = DVE_MAIN + 2 * (NTAIL - 1) + NQ + 2  # 40
    # ACT increments: 1 cast per tile
    # PE increments: one per tile 0..T-2, then one at the final stop matmul
    PE_DONE = T  # 18

    R_LAST = SCHED[-1]
    QF = R_LAST * N // NQ  # f32 elements per quarter of the last tile
    QR = R_LAST // NQ  # ray-blocks per quarter

    import contextlib

    with contextlib.ExitStack() as ctx:
        ec = ctx.enter_context
        w_sb = ec(nc.sbuf_tensor([P, NB * FREE], F32))
        m_sb = ec(nc.sbuf_tensor([P, NB * FREE], F32))
        s_sb = ec(nc.sbuf_tensor([P, NB * FREE], F32))
        # [wm | sw] interleaved per ray block: block r occupies columns
        # [r*2N, r*2N + 2N) of the slot, wm in the low half, sw in the high
        ws_sb = ec(nc.sbuf_tensor([P, NB * 2 * FREE], BF16))
        wb_sb = ec(nc.sbuf_tensor([P, NB * FREE], BF16))
        ai_sb = ec(nc.sbuf_tensor([P, 2 * N], F32))
        tr_sb = ec(nc.sbuf_tensor([P, 2 * N], F32))
        pad_sb = ec(nc.sbuf_tensor([P, 32], F32))
        tp_sb = ec(nc.sbuf_tensor([P, 32], F32))
        warm_sb = ec(nc.sbuf_tensor([P, 32], BF16))
        g12_ps = ec(nc.psum_tensor([P, 2 * N], F32))  # [W^T WM | W^T SW]
        slot_sem = [ec(nc.semaphore(f"dma_slot{i}")) for i in range(NB)]
        # tail: one semaphore per DMA so every wait is at a final total
        tw_sem = [ec(nc.semaphore(f"dma_w{j}")) for j in range(NTAIL)]
        tm_sem = [ec(nc.semaphore(f"dma_m{j}")) for j in range(NTAIL - 1)]
        ts_sem = [ec(nc.semaphore(f"dma_s{j}")) for j in range(NTAIL - 1)]
        tmq_sem = [ec(nc.semaphore(f"dma_mq{q}")) for q in range(NQ)]
        tsq_sem = [ec(nc.semaphore(f"dma_sq{q}")) for q in range(NQ)]
        ai_sem = ec(nc.semaphore("dma_ai"))
        act_sem = ec(nc.semaphore("act_sem"))
        gp_sem = ec(nc.semaphore("gp_sem"))
        dve_sem = ec(nc.semaphore("dve_sem"))
        pe_sem = ec(nc.semaphore("pe_sem"))
        block = ec(nc.Block(no_gpsimd_drain=True))

        def sl(i, n_el=None):
            base = (i % NB) * FREE
            return slice(base, base + (SCHED[i] * N if n_el is None else n_el))

        def f32_3d(t_sb, i, q=None):
            # [P, R, N] view of an io slot (or one quarter of the last slot)
            if q is None:
                return t_sb[:, sl(i)].rearrange("p (r n) -> p r n", n=N)
            base = (i % NB) * FREE
            return t_sb[:, base + q * QF : base + (q + 1) * QF].rearrange(
                "p (r n) -> p r n", n=N
            )

        def ws_3d(i, half, q=None):
            # [P, R, N] strided view into the [wm | sw] pair layout
            base2 = (i % NB) * 2 * FREE
            if q is None:
                r = SCHED[i]
                v = ws_sb[:, base2 : base2 + 2 * r * N]
            else:
                v = ws_sb[:, base2 + q * 2 * QF : base2 + (q + 1) * 2 * QF]
            v = v.rearrange("p (r x) -> p r x", x=2 * N)
            return v[:, :, half * N : (half + 1) * N]

        @block.sync
        def _(sync: bass.BassEngine):
            for i in range(TM):
                k = i % NB
                if i >= NB:
                    # io ring slot (i-NB): w consumed by ACT cast + DVE
                    # wm/sw; m/s consumed by DVE
                    sync.wait_ge(act_sem, i - NB + 1)
                    sync.wait_ge(dve_sem, 2 * (i - NB + 1))
                sync.dma_start(out=w_sb[:, sl(i)], in_=dram_view(w_h, i)).then_inc(
                    slot_sem[k], 16
                )
                sync.dma_start(out=m_sb[:, sl(i)], in_=dram_view(m_h, i)).then_inc(
                    slot_sem[k], 16
                )
                sync.dma_start(out=s_sb[:, sl(i)], in_=dram_view(s_h, i)).then_inc(
                    slot_sem[k], 16
                )
            # stream-major tail
            for j in range(NTAIL):
                i = TM + j
                sync.wait_ge(act_sem, i - NB + 1)
                sync.wait_ge(dve_sem, 2 * (i - NB + 1))
                sync.dma_start(out=w_sb[:, sl(i)], in_=dram_view(w_h, i)).then_inc(
                    tw_sem[j], 16
                )
                if j < NTAIL - 1:
                    sync.dma_start(out=m_sb[:, sl(i)], in_=dram_view(m_h, i)).then_inc(
                        tm_sem[j], 16
                    )
                    sync.dma_start(out=s_sb[:, sl(i)], in_=dram_view(s_h, i)).then_inc(
                        ts_sem[j], 16
                    )
            # final tile: m and s quartered; aimat dead last (the finale
            # needs it only after the last matmul)
            i = T - 1
            base = (i % NB) * FREE
            m_last = dram_view(m_h, i)
            s_last = dram_view(s_h, i)
            for q in range(NQ):
                sync.dma_start(
                    out=m_sb[:, base + q * QF : base + (q + 1) * QF],
                    in_=m_last[:, q * QF : (q + 1) * QF],
                ).then_inc(tmq_sem[q], 16)
            for q in range(NQ):
                sync.dma_start(
                    out=s_sb[:, base + q * QF : base + (q + 1) * QF],
                    in_=s_last[:, q * QF : (q + 1) * QF],
                ).then_inc(tsq_sem[q], 16)
            sync.dma_start(out=ai_sb[:], in_=ai_h[:, :]).then_inc(ai_sem, 16)
            sync.wait_ge(dve_sem, DVE_FINAL)
            sync.dma_start(out=out_h[:, :], in_=tp_sb[0:P:32, 0:32]).then_inc(
                pe_sem, 16
            )
            # the out-DMA must fully land before the NEFF ends: an in-flight
            # DMA across the NEFF boundary corrupts runtime state.
            sync.wait_ge(pe_sem, PE_DONE + 16)

        @block.scalar
        def _(scalar: bass.BassEngine):
            # warmup: first ACTIVATE triggers a ~5us ACT table load; eat it
            # here instead of on tile 0's cast (value is irrelevant; wb_sb
            # is only written later by this same engine)
            scalar.activation(
                warm_sb[:, :], wb_sb[:, 0:32], mybir.ActivationFunctionType.Copy
            )
            for i in range(TM):
                k = i % NB
                scalar.wait_ge(slot_sem[k], 48 * (i // NB + 1))
                if i >= NB:
                    # bf16 ring slot (i-NB) fully consumed by PE
                    scalar.wait_ge(pe_sem, i - NB + 1)
                scalar.activation(
                    wb_sb[:, sl(i)],
                    w_sb[:, sl(i)],
                    mybir.ActivationFunctionType.Copy,
                ).then_inc(act_sem, 1)
            for j in range(NTAIL):
                i = TM + j
                scalar.wait_ge(tw_sem[j], 16)
                scalar.wait_ge(pe_sem, i - NB + 1)
                scalar.activation(
                    wb_sb[:, sl(i)],
                    w_sb[:, sl(i)],
                    mybir.ActivationFunctionType.Copy,
                ).then_inc(act_sem, 1)

        @block.gpsimd
        def _(gpsimd: bass.BassEngine):
            # last tile's wm products, quarter by quarter (m stream chase)
            i = T - 1
            gpsimd.wait_ge(tw_sem[NTAIL - 1], 16)
            gpsimd.wait_ge(pe_sem, i - NB + 1)
            for q in range(NQ):
                gpsimd.wait_ge(tmq_sem[q], 16)
                gpsimd.tensor_mul(
                    ws_3d(i, 0, q), f32_3d(w_sb, i, q), f32_3d(m_sb, i, q)
                ).then_inc(gp_sem, 1)

        @block.vector
        def _(vector: bass.BassEngine):
            # zero the transpose staging pad once (cols 1..31 stay zero)
            vector.memset(pad_sb[:, :], 0.0)
            for i in range(TM):
                k = i % NB
                vector.wait_ge(slot_sem[k], 48 * (i // NB + 1))
                if i >= NB:
                    vector.wait_ge(pe_sem, i - NB + 1)
                vector.tensor_mul(
                    ws_3d(i, 0), f32_3d(w_sb, i), f32_3d(m_sb, i)
                ).then_inc(dve_sem, 1)
                vector.tensor_mul(
                    ws_3d(i, 1), f32_3d(s_sb, i), f32_3d(w_sb, i)
                ).then_inc(dve_sem, 1)
            for j in range(NTAIL - 1):
                i = TM + j
                vector.wait_ge(pe_sem, i - NB + 1)
                vector.wait_ge(tw_sem[j], 16)
                vector.wait_ge(tm_sem[j], 16)
                vector.tensor_mul(
                    ws_3d(i, 0), f32_3d(w_sb, i), f32_3d(m_sb, i)
                ).then_inc(dve_sem, 1)
                vector.wait_ge(ts_sem[j], 16)
                vector.tensor_mul(
                    ws_3d(i, 1), f32_3d(s_sb, i), f32_3d(w_sb, i)
                ).then_inc(dve_sem, 1)
            # last tile: chase the s quarters (wm quarters run on gpsimd)
            i = T - 1
            vector.wait_ge(pe_sem, i - NB + 1)
            vector.wait_ge(tw_sem[NTAIL - 1], 16)
            for q in range(NQ):
                vector.wait_ge(tsq_sem[q], 16)
                vector.tensor_mul(
                    ws_3d(i, 1, q), f32_3d(s_sb, i, q), f32_3d(w_sb, i, q)
                ).then_inc(dve_sem, 1)
            # finale: (G12 * [2A | I/3]) row-sum -> 4x32 fold.
            # drain: the stream-shuffle transpose does not interlock with
            # the preceding ALU op's writeback.
            vector.wait_ge(pe_sem, PE_DONE)
            vector.wait_ge(ai_sem, 16)
            vector.tensor_mul(tr_sb[:], g12_ps[:], ai_sb[:])
            vector.tensor_reduce(
                pad_sb[:, 0:1],
                tr_sb[:],
                axis=mybir.AxisListType.X,
                op=mybir.AluOpType.add,
            ).then_inc(dve_sem, 1)
            vector.drain()
            vector.transpose(out=tp_sb[:, :], in_=pad_sb[:, :]).then_inc(dve_sem, 1)

        @block.tensor
        def _(tensor: bass.BassEngine):
            def tile_mms(i, blocks, start=False, stop=False):
                base = (i % NB) * FREE
                base2 = (i % NB) * 2 * FREE
                last_mm = None
                for idx, r in enumerate(blocks):
                    wblk = slice(base + r * N, base + (r + 1) * N)
                    pblk = slice(base2 + r * 2 * N, base2 + (r + 1) * 2 * N)
                    last_mm = nc.tensor.matmul(
                        out=g12_ps[:],
                        lhsT=wb_sb[:, wblk],
                        rhs=ws_sb[:, pblk],
                        start=(start and idx == 0),
                        stop=(stop and idx == len(blocks) - 1),
                    )
                return last_mm

            for i in range(TM):
                tensor.wait_ge(act_sem, i + 1)
                tensor.wait_ge(dve_sem, 2 * i + 2)
                tile_mms(i, range(SCHED[i]), start=(i == 0)).then_inc(pe_sem, 1)
            for j in range(NTAIL - 1):
                i = TM + j
                tensor.wait_ge(act_sem, i + 1)
                tensor.wait_ge(dve_sem, DVE_MAIN + 2 * (j + 1))
                tile_mms(i, range(SCHED[i])).then_inc(pe_sem, 1)
            i = T - 1
            tensor.wait_ge(act_sem, T)
            last_mm = None
            for q in range(NQ):
                tensor.wait_ge(gp_sem, q + 1)
                tensor.wait_ge(dve_sem, DVE_MAIN + 2 * (NTAIL - 1) + q + 1)
                last_mm = tile_mms(
                    i, range(q * QR, (q + 1) * QR), stop=(q == NQ - 1)
                )
            last_mm.then_inc(pe_sem, 1)

    return nc


def _a2mat() -> np.ndarray:
    # transpose of (SL - SU): the kernel accumulates W^T WM = G1^T, and
    # <A, G1> = <A^T, G1^T>
    a = np.triu(np.ones((N, N), np.float32), 1) - np.tril(
        np.ones((N, N), np.float32), -1
    )
    return np.ascontiguousarray(a, dtype=np.float32)


def _aimat() -> np.ndarray:
    # loss weights folded in: [2A | I/3]
    return np.ascontiguousarray(
        np.concatenate(
            [2.0 * _a2mat(), np.eye(N, dtype=np.float32) / 3.0], axis=1
        )
    )


def kernel(weights: np.ndarray, distances: np.ndarray, intervals: np.ndarray):
    if "nc" not in _cached:
        _cached["nc"] = _build_nc()
    nc = _cached["nc"]

    w8 = np.ascontiguousarray(weights, np.float32).reshape(NCORES, B_PER, N)
    m8 = np.ascontiguousarray(distances, np.float32).reshape(NCORES, B_PER, N)
    s8 = np.ascontiguousarray(intervals, np.float32).reshape(NCORES, B_PER, N)
    ai = _aimat()

    in_maps = [
        {
            "weights": w8[i],
            "distances": m8[i],
            "intervals": s8[i],
            "aimat": ai,
        }
        for i in range(NCORES)
    ]
    res = run_bass_kernel_spmd(nc, in_maps, list(range(NCORES))).results

    total = 0.0
    for i in range(NCORES):
        total += res[i]["partials"].astype(np.float64).sum()

    loss = LOSS_WEIGHT * total / B
    return np.asarray(loss, dtype=np.float32)
